# revision 1
# baseline (speedup 1.0000x reference)
"""GCN block (3-hop symmetric-normalized propagation + LN/FFN/residual) on 8 trn2 cores.

Strategy:
  - Nodes sharded 8 ways (8192/core), edges partitioned by destination core.
  - Per core, destinations grouped into 64 blocks of 128 nodes; each block's
    edge list split by gather-table half (row < 32768 vs >= 32768) so that
    dma_gather's int16 indices always fit; both halves accumulate into the
    same PSUM result via a two-pass partial-sum scheme.
  - Messages gathered from a DRAM table (full normalized features, p-major
    permuted layout) with nc.gpsimd.dma_gather; scatter-add done with
    DVE-built one-hot matrices and PE matmuls accumulating in PSUM.
  - Between hops the updated owned block is AllGathered into every core's
    table (collective on TOPSP/SDMA, overlaps compute).
  - LN + FFN (x @ w1.T relu @ w2.T) node-local: batched LN on DVE, per-block
    PE transpose + 2 matmuls, residuals added back, outputs (out, r).
"""
import sys
sys.path.insert(0, '/opt/trn_rl_repo')
import os
import numpy as np

NC = 8          # cores
P = 128         # partitions
D = 64          # feature dim
HOPS = 3
LN_EPS = 1e-5
HALF = 32768    # int16-safe table split
CHUNK_TILES = 16  # 2048 edges per dma_gather call
LO, HI = 0, 1

_CACHE = {}


def _preprocess(N, edge_src, edge_dst, norm):
    """Partition/pad edges; returns per-core arrays + shared tile schedule."""
    NPC = N // NC          # nodes per core
    NB = NPC // P          # dst blocks per core
    E = edge_src.shape[0]

    s = edge_src.astype(np.int64)
    d = edge_dst.astype(np.int64)
    # table row for node n (p-major per core): k*NPC + (n%NPC)%128 * NB + (n%NPC)//128
    loc_s = s % NPC
    rho = (s // NPC) * NPC + (loc_s % P) * NB + (loc_s // P)
    k_d = d // NPC
    b_d = (d % NPC) // P
    dl_d = d % P
    half = (rho >= HALF).astype(np.int64)

    # group edges by (core, block, half)
    key = ((k_d * NB + b_d) * 2 + half).astype(np.int64)
    order = np.argsort(key, kind='stable')
    cnt = np.bincount(key, minlength=NC * NB * 2).reshape(NC, NB, 2)
    # equalized tile counts across cores (SPMD: one program)
    T = np.maximum(1, (cnt.max(axis=0) + P - 1) // P)  # [NB, 2] tiles
    if N <= HALF:
        T[:, HI] = 0
    TLO, THI = int(T[:, LO].sum()), int(T[:, HI].sum())
    TT = TLO + THI
    EP = TT * P  # padded edges per core

    rho_s = rho[order]
    dl_s = dl_d[order]
    starts = np.zeros(NC * NB * 2 + 1, np.int64)
    np.cumsum(np.bincount(key, minlength=NC * NB * 2), out=starts[1:])

    idx_all = np.zeros((NC, EP), np.int64)
    dstl_all = np.full((NC, EP), 200.0, np.float32)  # sentinel -> one-hot row of zeros
    # per-core streams: [all lo tiles b0..b63][all hi tiles b0..b63]
    tile_meta = []  # shared schedule: (half, block, first, last)
    for h in (LO, HI):
        for b in range(NB):
            for t in range(T[b, h]):
                tile_meta.append((h, b, t == 0, t == T[b, h] - 1))
    off_h = [0, TLO * P]
    for k in range(NC):
        for h in (LO, HI):
            pos = off_h[h]
            for b in range(NB):
                g = (k * NB + b) * 2 + h
                c = int(starts[g + 1] - starts[g])
                sl = slice(starts[g], starts[g + 1])
                idx_all[k, pos:pos + c] = rho_s[sl] - (HALF if h else 0)
                dstl_all[k, pos:pos + c] = dl_s[sl]
                # pad rows gather row 0 of the half; dstl stays sentinel
                pos += T[b, h] * P
    # wrapped int16 index layout: edge i -> [i%16, i//16], replicated to 128 partitions
    idx16 = idx_all.reshape(NC, EP // 16, 16).transpose(0, 2, 1).astype(np.int16)
    idx16 = np.tile(idx16, (1, 8, 1))  # [NC, 128, EP//16]
    dstl = dstl_all.reshape(NC, TT, P).transpose(0, 2, 1).copy()  # [NC, 128, TT]

    # gather-call schedule (shared): per half, runs of <=CHUNK_TILES tiles
    calls = []  # (half, tile0, ntiles)
    for h, t0, tn in ((LO, 0, TLO), (HI, TLO, THI)):
        t = 0
        while t < tn:
            n = min(CHUNK_TILES, tn - t)
            calls.append((h, t0 + t, n))
            t += n
    return dict(NPC=NPC, NB=NB, TT=TT, TLO=TLO, THI=THI, tile_meta=tile_meta,
                calls=calls, idx16=idx16, dstl=dstl)


def _build(N, pp):
    from concourse import bass, bacc, tile, mybir
    NPC, NB, TT = pp['NPC'], pp['NB'], pp['TT']
    EP = TT * P
    f32, i16 = mybir.dt.float32, mybir.dt.int16
    AO = mybir.AluOpType

    nc = bacc.Bacc("TRN2", target_bir_lowering=False, debug=False, num_devices=NC)
    # inputs (per-core)
    t_feat = nc.dram_tensor("feat", [P, NB * D], f32, kind="ExternalInput")
    t_idx = nc.dram_tensor("idx16", [P, EP // 16], i16, kind="ExternalInput")
    t_dstl = nc.dram_tensor("dstl", [P, TT], f32, kind="ExternalInput")
    t_norm = nc.dram_tensor("normv", [P, NB], f32, kind="ExternalInput")
    t_norm2 = nc.dram_tensor("norm2v", [P, NB], f32, kind="ExternalInput")
    t_iota = nc.dram_tensor("iotar", [P, P], f32, kind="ExternalInput")
    t_ident = nc.dram_tensor("ident", [P, P], f32, kind="ExternalInput")
    t_w1T = nc.dram_tensor("w1T", [D, D], f32, kind="ExternalInput")
    t_w2T = nc.dram_tensor("w2T", [D, D], f32, kind="ExternalInput")
    t_b1 = nc.dram_tensor("b1c", [D, 1], f32, kind="ExternalInput")
    t_b2 = nc.dram_tensor("b2b", [P, D], f32, kind="ExternalInput")
    t_gam = nc.dram_tensor("gamb", [P, D], f32, kind="ExternalInput")
    t_bet = nc.dram_tensor("betb", [P, D], f32, kind="ExternalInput")
    # outputs
    t_out = nc.dram_tensor("outp", [P, NB * D], f32, kind="ExternalOutput")
    t_r = nc.dram_tensor("routp", [P, NB * D], f32, kind="ExternalOutput")

    with tile.TileContext(nc) as tc:
        with tc.tile_pool(name="const", bufs=1) as cp, \
             tc.tile_pool(name="work", bufs=1) as wp, \
             tc.tile_pool(name="g", bufs=4) as gp, \
             tc.tile_pool(name="oh", bufs=3) as op_, \
             tc.tile_pool(name="ps", bufs=2, space="PSUM") as ps, \
             tc.tile_pool(name="dram", bufs=1, space="DRAM") as dr:

            # --- load constants / inputs into SBUF
            feat = cp.tile([P, NB * D], f32)
            idxt = cp.tile([P, EP // 16], i16)
            dstl = cp.tile([P, TT], f32)
            nrm = cp.tile([P, NB], f32)
            nrm2 = cp.tile([P, NB], f32)
            iot = cp.tile([P, P], f32)
            idn = cp.tile([P, P], f32)
            w1T = cp.tile([D, D], f32)
            w2T = cp.tile([D, D], f32)
            b1 = cp.tile([D, 1], f32)
            b2b = cp.tile([P, D], f32)
            gmb = cp.tile([P, D], f32)
            btb = cp.tile([P, D], f32)
            for tl, th in ((feat, t_feat), (idxt, t_idx), (dstl, t_dstl),
                           (nrm, t_norm), (nrm2, t_norm2), (iot, t_iota),
                           (idn, t_ident), (w1T, t_w1T), (w2T, t_w2T),
                           (b1, t_b1), (b2b, t_b2), (gmb, t_gam), (btb, t_bet)):
                nc.sync.dma_start(out=tl[:], in_=th[:])

            nh = wp.tile([P, NB * D], f32)     # owned block, premultiplied
            prt = wp.tile([P, NB * D], f32)    # partial agg between lo/hi passes
            ag_in = dr.tile([P, NB * D], f32)
            tables = []
            for hh in range(HOPS):
                tbl = dr.tile([N, D], f32, addr_space="Shared", tag=f"table{hh}")
                tables.append(tbl)

            def bs(b):
                return slice(b * D, (b + 1) * D)

            # nh0 = norm * features
            for b in range(NB):
                nc.vector.tensor_scalar(out=nh[:, bs(b)], in0=feat[:, bs(b)],
                                        scalar1=nrm[:, b:b + 1], scalar2=None,
                                        op0=AO.mult)

            rg = [list(range(NC))]
            for hop in range(1, HOPS + 1):
                # publish owned block -> table (all cores)
                table = tables[hop - 1]
                nc.sync.dma_start(out=ag_in[:], in_=nh[:])
                nc.gpsimd.collective_compute("AllGather", AO.bypass, replica_groups=rg,
                                             ins=[ag_in[:]], outs=[table[:]])
                scale = nrm2 if hop < HOPS else nrm
                acc = None
                for (h, tile0, ntl) in pp['calls']:
                    g = gp.tile([P, CHUNK_TILES, D], f32, tag="g")
                    base = table[HALF:, :] if h == HI else (table[:HALF, :] if N > HALF else table[:, :])
                    nc.gpsimd.dma_gather(
                        out_ap=g[:, :ntl, :], in_ap=base,
                        idxs_ap=idxt[:, tile0 * 8:(tile0 + ntl) * 8],
                        num_idxs=ntl * P, num_idxs_reg=ntl * P, elem_size=D,
                        single_packet=False)
                    ohb = op_.tile([P, CHUNK_TILES * P], f32, tag="oh")
                    nc.vector.tensor_tensor(
                        out=ohb[:, :ntl * P].rearrange("p (t n) -> p t n", n=P),
                        in0=iot[:].rearrange("p (o n) -> p o n", o=1)
                            .to_broadcast([P, ntl, P]),
                        in1=dstl[:, tile0:tile0 + ntl]
                            .rearrange("p (t o) -> p t o", o=1)
                            .to_broadcast([P, ntl, P]),
                        op=AO.is_equal)
                    for j in range(ntl):
                        tglob = tile0 + j
                        th, b, first, last = pp['tile_meta'][tglob]
                        if first:
                            acc = ps.tile([P, D], f32, tag="acc", space="PSUM")
                        nc.tensor.matmul(out=acc[:], lhsT=ohb[:, j * P:(j + 1) * P],
                                         rhs=g[:, j, :], start=first, stop=last)
                        if last:
                            if th == LO and pp['THI'] > 0:
                                nc.vector.tensor_copy(out=prt[:, bs(b)], in_=acc[:])
                            else:
                                if pp['THI'] > 0:
                                    nc.vector.tensor_tensor(out=nh[:, bs(b)], in0=acc[:],
                                                            in1=prt[:, bs(b)], op=AO.add)
                                else:
                                    nc.vector.tensor_copy(out=nh[:, bs(b)], in_=acc[:])
                                nc.vector.tensor_scalar(out=nh[:, bs(b)], in0=nh[:, bs(b)],
                                                        scalar1=scale[:, b:b + 1],
                                                        scalar2=None, op0=AO.mult)

            # --- nh now holds r = norm * agg. LayerNorm (batched) + FFN.
            r3 = nh[:].rearrange("p (b d) -> p b d", d=D)
            xc = wp.tile([P, NB * D], f32)
            xc3 = xc[:].rearrange("p (b d) -> p b d", d=D)
            sq = wp.tile([P, NB * D], f32)
            sq3 = sq[:].rearrange("p (b d) -> p b d", d=D)
            mu = wp.tile([P, NB], f32)
            ssq = wp.tile([P, NB], f32)
            rstd = wp.tile([P, NB], f32)
            X = mybir.AxisListType.X
            nc.vector.tensor_reduce(out=mu[:], in_=r3, axis=X, op=AO.add)
            nc.vector.tensor_scalar(out=mu[:], in0=mu[:], scalar1=1.0 / D, scalar2=None,
                                    op0=AO.mult)
            nc.vector.tensor_tensor(out=xc3, in0=r3,
                                    in1=mu[:].rearrange("p (b o) -> p b o", o=1).to_broadcast([P, NB, D]),
                                    op=AO.subtract)
            nc.vector.tensor_tensor(out=sq3, in0=xc3, in1=xc3, op=AO.mult)
            nc.vector.tensor_reduce(out=ssq[:], in_=sq3, axis=X, op=AO.add)
            nc.vector.tensor_scalar(out=ssq[:], in0=ssq[:], scalar1=1.0 / D, scalar2=None,
                                    op0=AO.mult)
            nc.vector.tensor_scalar(out=ssq[:], in0=ssq[:], scalar1=LN_EPS,
                                    scalar2=None, op0=AO.add)
            nc.scalar.activation(out=ssq[:], in_=ssq[:],
                                 func=mybir.ActivationFunctionType.Sqrt)
            nc.vector.reciprocal(rstd[:], ssq[:])
            # xln = xc * rstd * gamma + beta   (reuse xc buffer)
            nc.vector.tensor_tensor(out=xc3, in0=xc3,
                                    in1=rstd[:].rearrange("p (b o) -> p b o", o=1).to_broadcast([P, NB, D]),
                                    op=AO.mult)
            nc.vector.tensor_tensor(out=xc3, in0=xc3,
                                    in1=gmb[:].rearrange("p (o d) -> p o d", o=1).to_broadcast([P, NB, D]),
                                    op=AO.mult)
            nc.vector.tensor_tensor(out=xc3, in0=xc3,
                                    in1=btb[:].rearrange("p (o d) -> p o d", o=1).to_broadcast([P, NB, D]),
                                    op=AO.add)

            out_own = wp.tile([P, NB * D], f32)
            for b in range(NB):
                xT_ps = ps.tile([D, P], f32, tag="tr", space="PSUM")
                nc.tensor.transpose(out=xT_ps[:], in_=xc[:, bs(b)], identity=idn[:])
                xT = op_.tile([D, P], f32, tag="xT")
                nc.scalar.copy(xT[:], xT_ps[:])
                h1_ps = ps.tile([D, P], f32, tag="h1", space="PSUM")
                nc.tensor.matmul(out=h1_ps[:], lhsT=w1T[:], rhs=xT[:], start=True, stop=True)
                h1 = op_.tile([D, P], f32, tag="h1s")
                nc.scalar.activation(out=h1[:], in_=h1_ps[:],
                                     func=mybir.ActivationFunctionType.Relu,
                                     bias=b1[:, 0:1])
                ff_ps = ps.tile([P, D], f32, tag="ff", space="PSUM")
                nc.tensor.matmul(out=ff_ps[:], lhsT=h1[:], rhs=w2T[:], start=True, stop=True)
                nc.vector.tensor_tensor(out=out_own[:, bs(b)], in0=ff_ps[:],
                                        in1=nh[:, bs(b)], op=AO.add)
            o3 = out_own[:].rearrange("p (b d) -> p b d", d=D)
            nc.vector.tensor_tensor(out=o3, in0=o3,
                                    in1=feat[:].rearrange("p (b d) -> p b d", d=D), op=AO.add)
            nc.vector.tensor_tensor(out=o3, in0=o3,
                                    in1=b2b[:].rearrange("p (o d) -> p o d", o=1).to_broadcast([P, NB, D]),
                                    op=AO.add)
            nc.sync.dma_start(out=t_out[:], in_=out_own[:])
            nc.sync.dma_start(out=t_r[:], in_=nh[:])
    nc.compile()
    return nc


def kernel(features, edge_src, edge_dst, w1, b1, w2, b2, gamma, beta):
    from concourse import bass_utils
    features = np.asarray(features, np.float32)
    edge_src = np.asarray(edge_src, np.int32)
    edge_dst = np.asarray(edge_dst, np.int32)
    N = features.shape[0]
    NPC = N // NC
    NB = NPC // P

    deg = np.bincount(edge_dst, minlength=N).astype(np.float32)
    norm = 1.0 / np.sqrt(np.maximum(deg, 1.0))

    ck = (edge_src.tobytes(), edge_dst.tobytes(), N)
    import hashlib
    h = hashlib.sha1()
    for x in ck[:2]:
        h.update(x)
    h.update(str(N).encode())
    key = h.hexdigest()
    if key not in _CACHE:
        pp = _preprocess(N, edge_src, edge_dst, norm)
        ncb = _build(N, pp)
        _CACHE[key] = (pp, ncb)
    pp, ncb = _CACHE[key]

    # per-core host arrays
    iota_np = np.tile(np.arange(P, dtype=np.float32), (P, 1))
    ident_np = np.eye(P, dtype=np.float32)
    w1T_np = np.ascontiguousarray(np.asarray(w1, np.float32).T)
    w2T_np = np.ascontiguousarray(np.asarray(w2, np.float32).T)
    b1_np = np.asarray(b1, np.float32).reshape(D, 1)
    b2b_np = np.tile(np.asarray(b2, np.float32)[None, :], (P, 1))
    gam_np = np.tile(np.asarray(gamma, np.float32)[None, :], (P, 1))
    bet_np = np.tile(np.asarray(beta, np.float32)[None, :], (P, 1))

    in_maps = []
    for k in range(NC):
        fo = features[k * NPC:(k + 1) * NPC].reshape(NB, P, D).transpose(1, 0, 2) \
            .reshape(P, NB * D).copy()
        no = norm[k * NPC:(k + 1) * NPC].reshape(NB, P).T.copy()
        in_maps.append({
            "feat": fo, "idx16": pp['idx16'][k], "dstl": pp['dstl'][k],
            "normv": no, "norm2v": (no * no), "iotar": iota_np, "ident": ident_np,
            "w1T": w1T_np, "w2T": w2T_np, "b1c": b1_np, "b2b": b2b_np,
            "gamb": gam_np, "betb": bet_np,
        })

    trace = os.environ.get("GCN_TRACE", "0") == "1"
    res = bass_utils.run_bass_kernel_spmd(ncb, in_maps, core_ids=list(range(NC)),
                                          trace=trace)
    if trace and res.exec_time_ns is not None:
        print(f"HW exec time: {res.exec_time_ns} ns")

    out = np.empty((N, D), np.float32)
    r = np.empty((N, D), np.float32)
    for k in range(NC):
        o = res.results[k]["outp"].reshape(P, NB, D).transpose(1, 0, 2).reshape(NPC, D)
        rr = res.results[k]["routp"].reshape(P, NB, D).transpose(1, 0, 2).reshape(NPC, D)
        out[k * NPC:(k + 1) * NPC] = o
        r[k * NPC:(k + 1) * NPC] = rr
    return (out, r)



# revision 19
# speedup vs baseline: 2.5228x; 2.5228x over previous
"""GCN block (3-hop symmetric-normalized propagation + LN/FFN/residual) on 8 trn2 cores.

Strategy:
  - Nodes sharded 8 ways (8192/core), edges partitioned by destination core.
  - Per core, destinations grouped into 64 blocks of 128 nodes; each block's
    edge list split by gather-table half (row < 32768 vs >= 32768) so that
    dma_gather's int16 indices always fit; both halves accumulate into the
    same PSUM result via a two-pass partial-sum scheme.
  - Gather table is bf16 with 256B rows (64 feats + 64 pad) in shared DRAM.
    Messages gathered with nc.gpsimd.dma_gather in prepare_only mode,
    round-robined over all 4 SWDGE queues (4 independent DMA rings), each
    triggered immediately; scatter-add done with DVE-built bf16 one-hot
    matrices and bf16 PE matmuls accumulating in fp32 PSUM.
  - Between hops the updated owned block is cast to bf16 and AllGathered into
    every core's table (collective overlaps little; it is cheap).
  - LN in fp32 on DVE; FFN matmuls in bf16 (x @ w1.T relu @ w2.T), residuals
    fp32, outputs (out, r).
"""
import sys
sys.path.insert(0, '/opt/trn_rl_repo')
import os
import numpy as np

NC = 8          # cores
P = 128         # partitions
D = 64          # feature dim
ROW = 128       # padded bf16 row elems (256B)
HOPS = 3
LN_EPS = 1e-5
HALF = 32768    # int16-safe table split
CHUNK_TILES = 16  # 2048 edges per dma_gather call
NQ = 4          # SWDGE queues
LO, HI = 0, 1

_CACHE = {}


def _rebalance(N, edge_src, edge_dst):
    """Assign nodes to (core, block) bins balancing per-bin (lo, hi) edge
    counts, so per-block tile counts hit their floor. A node keeps its table
    half (orig id < HALF <=> cores 0..NC/2), so every edge's half bit is
    invariant under the reassignment. Returns (k_of, b_of, p_of, nodemap)."""
    NPC = N // NC
    NB = NPC // P
    if N <= HALF:
        groups = [(0, N, 0, NC)]
    else:
        groups = [(0, HALF, 0, NC // 2), (HALF, N, NC // 2, NC)]
    lo = np.bincount(edge_dst[edge_src < HALF], minlength=N).astype(np.int64)
    hi = np.bincount(edge_dst[edge_src >= HALF], minlength=N).astype(np.int64)
    k_of = np.empty(N, np.int64)
    b_of = np.empty(N, np.int64)
    p_of = np.empty(N, np.int64)
    nodemap = np.empty((NC, NPC), np.int64)
    for (n0, n1, c0, c1) in groups:
        ncores = c1 - c0
        nbins = ncores * NB
        nodes = np.arange(n0, n1)
        order = nodes[np.argsort(-(lo[nodes] + hi[nodes]), kind='stable')]
        binlo = np.zeros(nbins, np.int64)
        binhi = np.zeros(nbins, np.int64)
        slots = np.zeros(nbins, np.int64)

        def place(n, g):
            k = c0 + g // NB
            b = g % NB
            p = slots[g]
            k_of[n] = k
            b_of[n] = b
            p_of[n] = p
            nodemap[k, b * P + p] = n
            binlo[g] += lo[n]
            binhi[g] += hi[n]
            slots[g] += 1

        # The heaviest nodes are concentrated into the last NDB blocks ("dump"
        # bins): slot packing is exact (128 nodes per bin), so spare capacity
        # for the other blocks only exists if dump blocks absorb extra degree.
        NDB = 2
        ndump = ncores * P * NDB
        for i, n in enumerate(order[:ndump]):
            place(n, (i % (ncores * NDB)) // NDB * NB + (NB - 1 - i % NDB))
        # Remaining nodes: dealt one-per-bin per round (slot counts stay
        # exactly equal); within a round the bin furthest below the running
        # lo (or hi, alternating) average receives the node richest in it.
        live = np.nonzero(~((np.arange(nbins) % NB) >= (NB - NDB)))[0]
        rest = order[ndump:]
        nlive = len(live)
        for r in range(P):
            chunk = rest[r * nlive:(r + 1) * nlive]
            key = lo[chunk] if r % 2 == 0 else hi[chunk]
            nsort = chunk[np.argsort(-(2 * key + lo[chunk] + hi[chunk]),
                                     kind='stable')]
            bload = binlo[live] if r % 2 == 0 else binhi[live]
            bsort = live[np.argsort(bload + 0.25 * (binlo + binhi)[live],
                                    kind='stable')]
            for n, g in zip(nsort, bsort):
                place(n, int(g))
    return k_of, b_of, p_of, nodemap


def _preprocess(N, edge_src, edge_dst, norm):
    """Partition/pad edges; returns per-core arrays + shared tile schedule."""
    NPC = N // NC          # nodes per core
    NB = NPC // P          # dst blocks per core
    E = edge_src.shape[0]

    k_of, b_of, p_of, nodemap = _rebalance(N, edge_src, edge_dst)
    s = edge_src.astype(np.int64)
    d = edge_dst.astype(np.int64)
    # table row of node n (p-major per core): k*NPC + p*NB + b
    rho = k_of[s] * NPC + p_of[s] * NB + b_of[s]
    k_d = k_of[d]
    b_d = b_of[d]
    dl_d = p_of[d]
    half = (rho >= HALF).astype(np.int64)

    # group edges by (core, block, half)
    key = ((k_d * NB + b_d) * 2 + half).astype(np.int64)
    order = np.argsort(key, kind='stable')
    cnt = np.bincount(key, minlength=NC * NB * 2).reshape(NC, NB, 2)
    # equalized tile counts across cores (SPMD: one program)
    T = np.maximum(1, (cnt.max(axis=0) + P - 1) // P)  # [NB, 2] tiles
    if N <= HALF:
        T[:, HI] = 0
    TLO, THI = int(T[:, LO].sum()), int(T[:, HI].sum())
    TT = TLO + THI
    EP = TT * P  # padded edges per core

    rho_s = rho[order]
    dl_s = dl_d[order]
    starts = np.zeros(NC * NB * 2 + 1, np.int64)
    np.cumsum(np.bincount(key, minlength=NC * NB * 2), out=starts[1:])

    idx_all = np.zeros((NC, EP), np.int64)
    dstl_all = np.full((NC, EP), 200.0, np.float32)  # sentinel -> one-hot row of zeros
    # per-core streams: [all lo tiles b0..b63][all hi tiles b0..b63]
    tile_meta = []  # shared schedule: (half, block, first, last)
    for h in (LO, HI):
        for b in range(NB):
            for t in range(T[b, h]):
                tile_meta.append((h, b, t == 0, t == T[b, h] - 1))
    off_h = [0, TLO * P]
    for k in range(NC):
        for h in (LO, HI):
            pos = off_h[h]
            for b in range(NB):
                g = (k * NB + b) * 2 + h
                c = int(starts[g + 1] - starts[g])
                sl = slice(starts[g], starts[g + 1])
                idx_all[k, pos:pos + c] = rho_s[sl] - (HALF if h else 0)
                dstl_all[k, pos:pos + c] = dl_s[sl]
                # pad rows gather row 0 of the half; dstl stays sentinel
                pos += T[b, h] * P
    # wrapped int16 index layout: edge i -> [i%16, i//16], replicated to 128 partitions
    idx16 = idx_all.reshape(NC, EP // 16, 16).transpose(0, 2, 1).astype(np.int16)
    idx16 = np.tile(idx16, (1, 8, 1))  # [NC, 128, EP//16]
    dstl = dstl_all.reshape(NC, TT, P).transpose(0, 2, 1).copy()  # [NC, 128, TT]

    # gather-call schedule (shared): per half, runs of <=CHUNK_TILES tiles
    calls = []  # (half, tile0, ntiles)
    for h, t0, tn in ((LO, 0, TLO), (HI, TLO, THI)):
        t = 0
        while t < tn:
            n = min(CHUNK_TILES, tn - t)
            calls.append((h, t0 + t, n))
            t += n
    return dict(NPC=NPC, NB=NB, TT=TT, TLO=TLO, THI=THI, tile_meta=tile_meta,
                calls=calls, idx16=idx16, dstl=dstl, nodemap=nodemap)


def _build(N, pp):
    from concourse import bass, bacc, tile, mybir
    NPC, NB, TT = pp['NPC'], pp['NB'], pp['TT']
    EP = TT * P
    f32, bf16, i16 = mybir.dt.float32, mybir.dt.bfloat16, mybir.dt.int16
    AO = mybir.AluOpType

    nc = bacc.Bacc("TRN2", target_bir_lowering=False, debug=False, num_devices=NC,
                   num_swdge_queues=NQ)
    # inputs (per-core)
    t_feat = nc.dram_tensor("feat", [P, NB * D], f32, kind="ExternalInput")
    t_tab0 = nc.dram_tensor("tab0", [P, NB * ROW], bf16, kind="ExternalInput")
    t_idx = nc.dram_tensor("idx16", [P, EP // 16], i16, kind="ExternalInput")
    t_dstl = nc.dram_tensor("dstl", [P, TT], bf16, kind="ExternalInput")
    t_norm = nc.dram_tensor("normv", [P, NB], f32, kind="ExternalInput")
    t_norm2 = nc.dram_tensor("norm2v", [P, NB], f32, kind="ExternalInput")
    t_iota = nc.dram_tensor("iotar", [P, P], bf16, kind="ExternalInput")
    t_ident = nc.dram_tensor("ident", [P, P], bf16, kind="ExternalInput")
    t_w1T = nc.dram_tensor("w1T", [D, D], bf16, kind="ExternalInput")
    t_w2T = nc.dram_tensor("w2T", [D, D], bf16, kind="ExternalInput")
    t_b1 = nc.dram_tensor("b1c", [D, 1], f32, kind="ExternalInput")
    t_b2 = nc.dram_tensor("b2b", [P, D], f32, kind="ExternalInput")
    t_gam = nc.dram_tensor("gamb", [P, D], f32, kind="ExternalInput")
    t_bet = nc.dram_tensor("betb", [P, D], f32, kind="ExternalInput")
    # outputs
    t_out = nc.dram_tensor("outp", [P, NB * D], f32, kind="ExternalOutput")
    t_r = nc.dram_tensor("routp", [P, NB * D], f32, kind="ExternalOutput")

    with tile.TileContext(nc) as tc:
        with tc.tile_pool(name="const", bufs=1) as cp, \
             tc.tile_pool(name="work", bufs=1) as wp, \
             tc.tile_pool(name="g", bufs=8) as gp, \
             tc.tile_pool(name="oh", bufs=4) as op_, \
             tc.tile_pool(name="ps", bufs=3, space="PSUM") as ps, \
             tc.tile_pool(name="psf", bufs=1, space="PSUM") as psf, \
             tc.tile_pool(name="dram", bufs=1, space="DRAM") as dr:

            # --- load constants / inputs into SBUF
            feat = cp.tile([P, NB * D], f32)
            idxt = cp.tile([P, EP // 16], i16)
            dstl = cp.tile([P, TT], bf16)
            nrm = cp.tile([P, NB], f32)
            nrm2 = cp.tile([P, NB], f32)
            iot = cp.tile([P, P], bf16)
            idn = cp.tile([P, P], bf16)
            w1T = cp.tile([D, D], bf16)
            w2T = cp.tile([D, D], bf16)
            b1 = cp.tile([D, 1], f32)
            b2b = cp.tile([P, D], f32)
            gmb = cp.tile([P, D], f32)
            btb = cp.tile([P, D], f32)
            for tl, th in ((feat, t_feat), (idxt, t_idx), (dstl, t_dstl),
                           (nrm, t_norm), (nrm2, t_norm2), (iot, t_iota),
                           (idn, t_ident), (w1T, t_w1T), (w2T, t_w2T),
                           (b1, t_b1), (b2b, t_b2), (gmb, t_gam), (btb, t_bet)):
                nc.sync.dma_start(out=tl[:], in_=th[:])

            nh = wp.tile([P, NB * D], f32)     # owned block, premultiplied
            prt = wp.tile([P, NB * D], f32)    # partial agg between lo/hi passes
            agb = wp.tile([P, NB * ROW], bf16)  # bf16 padded staging for table
            nc.vector.memset(agb[:], 0.0)
            ag_in = dr.tile([P, NB * ROW], bf16)
            tables = []
            for hh in range(HOPS):
                tbl = dr.tile([N, ROW], bf16, addr_space="Shared", tag=f"table{hh}")
                tables.append(tbl)

            def bs(b):
                return slice(b * D, (b + 1) * D)

            agb3 = agb[:].rearrange("p (b c) -> p b c", c=ROW)
            nh3 = nh[:].rearrange("p (b d) -> p b d", d=D)

            rg = [list(range(NC))]
            qi = 0
            for hop in range(1, HOPS + 1):
                # publish owned block -> table (all cores), bf16 padded rows.
                # hop 1's slice (norm*features) is precomputed on the host.
                table = tables[hop - 1]
                if hop == 1:
                    nc.sync.dma_start(out=ag_in[:], in_=t_tab0[:])
                    nc.gpsimd.collective_compute("AllGather", AO.bypass,
                                                 replica_groups=rg,
                                                 ins=[ag_in[:]], outs=[table[:]])
                else:
                    nc.scalar.copy(out=agb3[:, :, 0:D], in_=nh3)
                    nc.sync.dma_start(out=ag_in[:], in_=agb[:])
                    nc.gpsimd.collective_compute("AllGather", AO.bypass,
                                                 replica_groups=rg,
                                                 ins=[ag_in[:]], outs=[table[:]])
                scale = nrm2 if hop < HOPS else nrm
                acc = None
                for (h, tile0, ntl) in pp['calls']:
                    g = gp.tile([P, CHUNK_TILES, ROW], bf16, tag="g")
                    base = table[HALF:, :] if h == HI else (table[:HALF, :] if N > HALF else table[:, :])
                    q = qi % NQ
                    qi += 1
                    nc.gpsimd.dma_gather(
                        out_ap=g[:, :ntl, :], in_ap=base,
                        idxs_ap=idxt[:, tile0 * 8:(tile0 + ntl) * 8],
                        num_idxs=ntl * P, num_idxs_reg=ntl * P, elem_size=ROW,
                        single_packet=False, queue_num=q)
                    ohb = op_.tile([P, CHUNK_TILES * P], bf16, tag="oh")
                    nc.vector.tensor_tensor(
                        out=ohb[:, :ntl * P].rearrange("p (t n) -> p t n", n=P),
                        in0=iot[:].rearrange("p (o n) -> p o n", o=1)
                            .to_broadcast([P, ntl, P]),
                        in1=dstl[:, tile0:tile0 + ntl]
                            .rearrange("p (t o) -> p t o", o=1)
                            .to_broadcast([P, ntl, P]),
                        op=AO.is_equal)
                    for j in range(ntl):
                        tglob = tile0 + j
                        th, b, first, last = pp['tile_meta'][tglob]
                        if first:
                            acc = ps.tile([P, D], f32, tag="acc", space="PSUM")
                        nc.tensor.matmul(out=acc[:], lhsT=ohb[:, j * P:(j + 1) * P],
                                         rhs=g[:, j, 0:D], start=first, stop=last)
                        if last:
                            if th == LO and pp['THI'] > 0:
                                nc.vector.tensor_copy(out=prt[:, bs(b)], in_=acc[:])
                            else:
                                if pp['THI'] > 0:
                                    nc.vector.tensor_tensor(out=nh[:, bs(b)], in0=acc[:],
                                                            in1=prt[:, bs(b)], op=AO.add)
                                else:
                                    nc.vector.tensor_copy(out=nh[:, bs(b)], in_=acc[:])
                                nc.vector.tensor_scalar(out=nh[:, bs(b)], in0=nh[:, bs(b)],
                                                        scalar1=scale[:, b:b + 1],
                                                        scalar2=None, op0=AO.mult)

            # --- nh now holds r = norm * agg. LayerNorm (batched) + FFN.
            r3 = nh[:].rearrange("p (b d) -> p b d", d=D)
            xc = wp.tile([P, NB * D], f32)
            xc3 = xc[:].rearrange("p (b d) -> p b d", d=D)
            sq3 = prt[:].rearrange("p (b d) -> p b d", d=D)  # reuse prt as scratch
            mu = wp.tile([P, NB], f32)
            ssq = wp.tile([P, NB], f32)
            rstd = wp.tile([P, NB], f32)
            X = mybir.AxisListType.X
            nc.vector.tensor_reduce(out=mu[:], in_=r3, axis=X, op=AO.add)
            nc.vector.tensor_scalar(out=mu[:], in0=mu[:], scalar1=1.0 / D, scalar2=None,
                                    op0=AO.mult)
            nc.vector.tensor_tensor(out=xc3, in0=r3,
                                    in1=mu[:].rearrange("p (b o) -> p b o", o=1).to_broadcast([P, NB, D]),
                                    op=AO.subtract)
            nc.vector.tensor_tensor(out=sq3, in0=xc3, in1=xc3, op=AO.mult)
            nc.vector.tensor_reduce(out=ssq[:], in_=sq3, axis=X, op=AO.add)
            nc.vector.tensor_scalar(out=ssq[:], in0=ssq[:], scalar1=1.0 / D, scalar2=None,
                                    op0=AO.mult)
            nc.vector.tensor_scalar(out=ssq[:], in0=ssq[:], scalar1=LN_EPS,
                                    scalar2=None, op0=AO.add)
            nc.scalar.activation(out=ssq[:], in_=ssq[:],
                                 func=mybir.ActivationFunctionType.Sqrt)
            nc.vector.reciprocal(rstd[:], ssq[:])
            # xln = xc * rstd * gamma + beta   (reuse xc buffer)
            nc.vector.tensor_tensor(out=xc3, in0=xc3,
                                    in1=rstd[:].rearrange("p (b o) -> p b o", o=1).to_broadcast([P, NB, D]),
                                    op=AO.mult)
            nc.vector.tensor_tensor(out=xc3, in0=xc3,
                                    in1=gmb[:].rearrange("p (o d) -> p o d", o=1).to_broadcast([P, NB, D]),
                                    op=AO.mult)
            nc.vector.tensor_tensor(out=xc3, in0=xc3,
                                    in1=btb[:].rearrange("p (o d) -> p o d", o=1).to_broadcast([P, NB, D]),
                                    op=AO.add)
            # bf16 copy of the LN output for the PE matmuls
            xcb = wp.tile([P, NB * D], bf16)
            nc.scalar.copy(out=xcb[:], in_=xc[:])

            out_own = wp.tile([P, NB * D], f32)
            for b in range(NB):
                xT_ps = psf.tile([D, P], bf16, tag="tr", space="PSUM")
                nc.tensor.transpose(out=xT_ps[:], in_=xcb[:, bs(b)], identity=idn[:])
                xT = op_.tile([D, P], bf16, tag="xT")
                nc.scalar.copy(xT[:], xT_ps[:])
                h1_ps = psf.tile([D, P], f32, tag="h1", space="PSUM")
                nc.tensor.matmul(out=h1_ps[:], lhsT=w1T[:], rhs=xT[:], start=True, stop=True)
                h1 = op_.tile([D, P], bf16, tag="h1s")
                nc.scalar.activation(out=h1[:], in_=h1_ps[:],
                                     func=mybir.ActivationFunctionType.Relu,
                                     bias=b1[:, 0:1])
                ff_ps = psf.tile([P, D], f32, tag="ff", space="PSUM")
                nc.tensor.matmul(out=ff_ps[:], lhsT=h1[:], rhs=w2T[:], start=True, stop=True)
                nc.vector.tensor_tensor(out=out_own[:, bs(b)], in0=ff_ps[:],
                                        in1=nh[:, bs(b)], op=AO.add)
            o3 = out_own[:].rearrange("p (b d) -> p b d", d=D)
            nc.vector.tensor_tensor(out=o3, in0=o3,
                                    in1=feat[:].rearrange("p (b d) -> p b d", d=D), op=AO.add)
            nc.vector.tensor_tensor(out=o3, in0=o3,
                                    in1=b2b[:].rearrange("p (o d) -> p o d", o=1).to_broadcast([P, NB, D]),
                                    op=AO.add)
            nc.sync.dma_start(out=t_out[:], in_=out_own[:])
            nc.sync.dma_start(out=t_r[:], in_=nh[:])
    nc.compile()
    return nc


def kernel(features, edge_src, edge_dst, w1, b1, w2, b2, gamma, beta):
    from concourse import bass_utils
    import ml_dtypes
    bf = ml_dtypes.bfloat16
    features = np.asarray(features, np.float32)
    edge_src = np.asarray(edge_src, np.int32)
    edge_dst = np.asarray(edge_dst, np.int32)
    N = features.shape[0]
    NPC = N // NC
    NB = NPC // P

    deg = np.bincount(edge_dst, minlength=N).astype(np.float32)
    norm = 1.0 / np.sqrt(np.maximum(deg, 1.0))

    import hashlib
    h = hashlib.sha1()
    h.update(edge_src.tobytes())
    h.update(edge_dst.tobytes())
    h.update(str(N).encode())
    key = h.hexdigest()
    if key not in _CACHE:
        pp = _preprocess(N, edge_src, edge_dst, norm)
        ncb = _build(N, pp)
        _CACHE[key] = (pp, ncb)
    pp, ncb = _CACHE[key]

    # per-core host arrays
    iota_np = np.tile(np.arange(P, dtype=np.float32), (P, 1)).astype(bf)
    ident_np = np.eye(P, dtype=np.float32).astype(bf)
    w1T_np = np.ascontiguousarray(np.asarray(w1, np.float32).T).astype(bf)
    w2T_np = np.ascontiguousarray(np.asarray(w2, np.float32).T).astype(bf)
    b1_np = np.asarray(b1, np.float32).reshape(D, 1)
    b2b_np = np.tile(np.asarray(b2, np.float32)[None, :], (P, 1))
    gam_np = np.tile(np.asarray(gamma, np.float32)[None, :], (P, 1))
    bet_np = np.tile(np.asarray(beta, np.float32)[None, :], (P, 1))

    in_maps = []
    for k in range(NC):
        nm = pp['nodemap'][k]
        fo = features[nm].reshape(NB, P, D).transpose(1, 0, 2) \
            .reshape(P, NB * D).copy()
        no = norm[nm].reshape(NB, P).T.copy()
        t0 = np.zeros((P, NB, ROW), np.float32)
        t0[:, :, :D] = (fo * np.repeat(no, D, 1)).reshape(P, NB, D)
        in_maps.append({
            "feat": fo, "tab0": t0.reshape(P, NB * ROW).astype(bf),
            "idx16": pp['idx16'][k], "dstl": pp['dstl'][k].astype(bf),
            "normv": no, "norm2v": (no * no), "iotar": iota_np, "ident": ident_np,
            "w1T": w1T_np, "w2T": w2T_np, "b1c": b1_np, "b2b": b2b_np,
            "gamb": gam_np, "betb": bet_np,
        })

    trace = os.environ.get("GCN_TRACE", "0") == "1"
    res = bass_utils.run_bass_kernel_spmd(ncb, in_maps, core_ids=list(range(NC)),
                                          trace=trace)
    if trace and res.exec_time_ns is not None:
        print(f"HW exec time: {res.exec_time_ns} ns")

    out = np.empty((N, D), np.float32)
    r = np.empty((N, D), np.float32)
    for k in range(NC):
        nm = pp['nodemap'][k]
        o = res.results[k]["outp"].reshape(P, NB, D).transpose(1, 0, 2).reshape(NPC, D)
        rr = res.results[k]["routp"].reshape(P, NB, D).transpose(1, 0, 2).reshape(NPC, D)
        out[nm] = o
        r[nm] = rr
    return (out, r)


# revision 20
# speedup vs baseline: 2.7205x; 1.0784x over previous
"""GCN block (3-hop symmetric-normalized propagation + LN/FFN/residual) on 8 trn2 cores.

Strategy:
  - Nodes sharded 8 ways (8192/core), edges partitioned by destination core.
  - Per core, destinations grouped into 64 blocks of 128 nodes; each block's
    edge list split by gather-table half (row < 32768 vs >= 32768) so that
    dma_gather's int16 indices always fit; both halves accumulate into the
    same PSUM result via a two-pass partial-sum scheme.
  - Gather table is bf16 with 256B rows (64 feats + 64 pad) in shared DRAM.
    Messages gathered with nc.gpsimd.dma_gather in prepare_only mode,
    round-robined over all 4 SWDGE queues (4 independent DMA rings), each
    triggered immediately; scatter-add done with DVE-built bf16 one-hot
    matrices and bf16 PE matmuls accumulating in fp32 PSUM.
  - Between hops the updated owned block is cast to bf16 and AllGathered into
    every core's table (collective overlaps little; it is cheap).
  - LN in fp32 on DVE; FFN matmuls in bf16 (x @ w1.T relu @ w2.T), residuals
    fp32, outputs (out, r).
"""
import sys
sys.path.insert(0, '/opt/trn_rl_repo')
import os
import numpy as np

NC = 8          # cores
P = 128         # partitions
D = 64          # feature dim
ROW = 128       # padded bf16 row elems (256B)
HOPS = 3
LN_EPS = 1e-5
HALF = 32768    # int16-safe table split
CHUNK_TILES = 16  # 2048 edges per dma_gather call
NQ = 4          # SWDGE queues
LO, HI = 0, 1

_CACHE = {}


def _rebalance(N, edge_src, edge_dst):
    """Assign nodes to (core, block) bins balancing per-bin (lo, hi) edge
    counts, so per-block tile counts hit their floor. A node keeps its table
    half (orig id < HALF <=> cores 0..NC/2), so every edge's half bit is
    invariant under the reassignment. Returns (k_of, b_of, p_of, nodemap)."""
    NPC = N // NC
    NB = NPC // P
    if N <= HALF:
        groups = [(0, N, 0, NC)]
    else:
        groups = [(0, HALF, 0, NC // 2), (HALF, N, NC // 2, NC)]
    lo = np.bincount(edge_dst[edge_src < HALF], minlength=N).astype(np.int64)
    hi = np.bincount(edge_dst[edge_src >= HALF], minlength=N).astype(np.int64)
    k_of = np.empty(N, np.int64)
    b_of = np.empty(N, np.int64)
    p_of = np.empty(N, np.int64)
    nodemap = np.empty((NC, NPC), np.int64)
    for (n0, n1, c0, c1) in groups:
        ncores = c1 - c0
        nbins = ncores * NB
        nodes = np.arange(n0, n1)
        order = nodes[np.argsort(-(lo[nodes] + hi[nodes]), kind='stable')]
        binlo = np.zeros(nbins, np.int64)
        binhi = np.zeros(nbins, np.int64)
        slots = np.zeros(nbins, np.int64)

        def place(n, g):
            k = c0 + g // NB
            b = g % NB
            p = slots[g]
            k_of[n] = k
            b_of[n] = b
            p_of[n] = p
            nodemap[k, b * P + p] = n
            binlo[g] += lo[n]
            binhi[g] += hi[n]
            slots[g] += 1

        # The heaviest nodes are concentrated into the last NDB blocks ("dump"
        # bins): slot packing is exact (128 nodes per bin), so spare capacity
        # for the other blocks only exists if dump blocks absorb extra degree.
        NDB = 1
        ndump = ncores * P * NDB
        for i, n in enumerate(order[:ndump]):
            place(n, (i % (ncores * NDB)) // NDB * NB + (NB - 1 - i % NDB))
        # Remaining nodes: dealt one-per-bin per round (slot counts stay
        # exactly equal); within a round the bin furthest below the running
        # lo (or hi, alternating) average receives the node richest in it.
        live = np.nonzero(~((np.arange(nbins) % NB) >= (NB - NDB)))[0]
        rest = order[ndump:]
        nlive = len(live)
        for r in range(P):
            chunk = rest[r * nlive:(r + 1) * nlive]
            key = lo[chunk] if r % 2 == 0 else hi[chunk]
            nsort = chunk[np.argsort(-(2 * key + lo[chunk] + hi[chunk]),
                                     kind='stable')]
            bload = binlo[live] if r % 2 == 0 else binhi[live]
            bsort = live[np.argsort(bload + 0.25 * (binlo + binhi)[live],
                                    kind='stable')]
            for n, g in zip(nsort, bsort):
                place(n, int(g))
    return k_of, b_of, p_of, nodemap


def _preprocess(N, edge_src, edge_dst, norm):
    """Partition/pad edges; returns per-core arrays + shared tile schedule."""
    NPC = N // NC          # nodes per core
    NB = NPC // P          # dst blocks per core
    E = edge_src.shape[0]

    k_of, b_of, p_of, nodemap = _rebalance(N, edge_src, edge_dst)
    s = edge_src.astype(np.int64)
    d = edge_dst.astype(np.int64)
    # table row of node n (p-major per core): k*NPC + p*NB + b
    rho = k_of[s] * NPC + p_of[s] * NB + b_of[s]
    k_d = k_of[d]
    b_d = b_of[d]
    dl_d = p_of[d]
    half = (rho >= HALF).astype(np.int64)

    # group edges by (core, block, half)
    key = ((k_d * NB + b_d) * 2 + half).astype(np.int64)
    order = np.argsort(key, kind='stable')
    cnt = np.bincount(key, minlength=NC * NB * 2).reshape(NC, NB, 2)
    # equalized tile counts across cores (SPMD: one program)
    T = np.maximum(1, (cnt.max(axis=0) + P - 1) // P)  # [NB, 2] tiles
    if N <= HALF:
        T[:, HI] = 0
    TLO, THI = int(T[:, LO].sum()), int(T[:, HI].sum())
    TT = TLO + THI
    EP = TT * P  # padded edges per core

    rho_s = rho[order]
    dl_s = dl_d[order]
    starts = np.zeros(NC * NB * 2 + 1, np.int64)
    np.cumsum(np.bincount(key, minlength=NC * NB * 2), out=starts[1:])

    idx_all = np.zeros((NC, EP), np.int64)
    dstl_all = np.full((NC, EP), 200.0, np.float32)  # sentinel -> one-hot row of zeros
    # per-core streams: [all lo tiles b0..b63][all hi tiles b0..b63]
    tile_meta = []  # shared schedule: (half, block, first, last)
    for h in (LO, HI):
        for b in range(NB):
            for t in range(T[b, h]):
                tile_meta.append((h, b, t == 0, t == T[b, h] - 1))
    off_h = [0, TLO * P]
    for k in range(NC):
        for h in (LO, HI):
            pos = off_h[h]
            for b in range(NB):
                g = (k * NB + b) * 2 + h
                c = int(starts[g + 1] - starts[g])
                sl = slice(starts[g], starts[g + 1])
                idx_all[k, pos:pos + c] = rho_s[sl] - (HALF if h else 0)
                dstl_all[k, pos:pos + c] = dl_s[sl]
                # pad rows gather row 0 of the half; dstl stays sentinel
                pos += T[b, h] * P
    # wrapped int16 index layout: edge i -> [i%16, i//16], replicated to 128 partitions
    idx16 = idx_all.reshape(NC, EP // 16, 16).transpose(0, 2, 1).astype(np.int16)
    idx16 = np.tile(idx16, (1, 8, 1))  # [NC, 128, EP//16]
    dstl = dstl_all.reshape(NC, TT, P).transpose(0, 2, 1).copy()  # [NC, 128, TT]

    # gather-call schedule (shared): per half, runs of <=CHUNK_TILES tiles
    calls = []  # (half, tile0, ntiles)
    for h, t0, tn in ((LO, 0, TLO), (HI, TLO, THI)):
        t = 0
        while t < tn:
            n = min(CHUNK_TILES, tn - t)
            calls.append((h, t0 + t, n))
            t += n
    return dict(NPC=NPC, NB=NB, TT=TT, TLO=TLO, THI=THI, tile_meta=tile_meta,
                calls=calls, idx16=idx16, dstl=dstl, nodemap=nodemap)


def _build(N, pp):
    from concourse import bass, bacc, tile, mybir
    NPC, NB, TT = pp['NPC'], pp['NB'], pp['TT']
    EP = TT * P
    f32, bf16, i16 = mybir.dt.float32, mybir.dt.bfloat16, mybir.dt.int16
    AO = mybir.AluOpType

    nc = bacc.Bacc("TRN2", target_bir_lowering=False, debug=False, num_devices=NC,
                   num_swdge_queues=NQ)
    # inputs (per-core)
    t_feat = nc.dram_tensor("feat", [P, NB * D], f32, kind="ExternalInput")
    t_tab0 = nc.dram_tensor("tab0", [P, NB * ROW], bf16, kind="ExternalInput")
    t_idx = nc.dram_tensor("idx16", [P, EP // 16], i16, kind="ExternalInput")
    t_dstl = nc.dram_tensor("dstl", [P, TT], bf16, kind="ExternalInput")
    t_norm = nc.dram_tensor("normv", [P, NB], f32, kind="ExternalInput")
    t_norm2 = nc.dram_tensor("norm2v", [P, NB], f32, kind="ExternalInput")
    t_iota = nc.dram_tensor("iotar", [P, P], bf16, kind="ExternalInput")
    t_ident = nc.dram_tensor("ident", [P, P], bf16, kind="ExternalInput")
    t_w1T = nc.dram_tensor("w1T", [D, D], bf16, kind="ExternalInput")
    t_w2T = nc.dram_tensor("w2T", [D, D], bf16, kind="ExternalInput")
    t_b1 = nc.dram_tensor("b1c", [D, 1], f32, kind="ExternalInput")
    t_b2 = nc.dram_tensor("b2b", [P, D], f32, kind="ExternalInput")
    t_gam = nc.dram_tensor("gamb", [P, D], f32, kind="ExternalInput")
    t_bet = nc.dram_tensor("betb", [P, D], f32, kind="ExternalInput")
    # outputs
    t_out = nc.dram_tensor("outp", [P, NB * D], f32, kind="ExternalOutput")
    t_r = nc.dram_tensor("routp", [P, NB * D], f32, kind="ExternalOutput")

    with tile.TileContext(nc) as tc:
        with tc.tile_pool(name="const", bufs=1) as cp, \
             tc.tile_pool(name="work", bufs=1) as wp, \
             tc.tile_pool(name="g", bufs=8) as gp, \
             tc.tile_pool(name="oh", bufs=4) as op_, \
             tc.tile_pool(name="ps", bufs=3, space="PSUM") as ps, \
             tc.tile_pool(name="psf", bufs=1, space="PSUM") as psf, \
             tc.tile_pool(name="dram", bufs=1, space="DRAM") as dr:

            # --- load constants / inputs into SBUF
            feat = cp.tile([P, NB * D], f32)
            idxt = cp.tile([P, EP // 16], i16)
            dstl = cp.tile([P, TT], bf16)
            nrm = cp.tile([P, NB], f32)
            nrm2 = cp.tile([P, NB], f32)
            iot = cp.tile([P, P], bf16)
            idn = cp.tile([P, P], bf16)
            w1T = cp.tile([D, D], bf16)
            w2T = cp.tile([D, D], bf16)
            b1 = cp.tile([D, 1], f32)
            b2b = cp.tile([P, D], f32)
            gmb = cp.tile([P, D], f32)
            btb = cp.tile([P, D], f32)
            for tl, th in ((feat, t_feat), (idxt, t_idx), (dstl, t_dstl),
                           (nrm, t_norm), (nrm2, t_norm2), (iot, t_iota),
                           (idn, t_ident), (w1T, t_w1T), (w2T, t_w2T),
                           (b1, t_b1), (b2b, t_b2), (gmb, t_gam), (btb, t_bet)):
                nc.sync.dma_start(out=tl[:], in_=th[:])

            nh = wp.tile([P, NB * D], f32)     # owned block, premultiplied
            prt = wp.tile([P, NB * D], f32)    # partial agg between lo/hi passes
            agb = wp.tile([P, NB * ROW], bf16)  # bf16 padded staging for table
            nc.vector.memset(agb[:], 0.0)
            ag_in = dr.tile([P, NB * ROW], bf16)
            tables = []
            for hh in range(HOPS):
                tbl = dr.tile([N, ROW], bf16, addr_space="Shared", tag=f"table{hh}")
                tables.append(tbl)

            def bs(b):
                return slice(b * D, (b + 1) * D)

            agb3 = agb[:].rearrange("p (b c) -> p b c", c=ROW)
            nh3 = nh[:].rearrange("p (b d) -> p b d", d=D)

            rg = [list(range(NC))]
            qi = 0
            for hop in range(1, HOPS + 1):
                # publish owned block -> table (all cores), bf16 padded rows.
                # hop 1's slice (norm*features) is precomputed on the host.
                table = tables[hop - 1]
                if hop == 1:
                    nc.sync.dma_start(out=ag_in[:], in_=t_tab0[:])
                    nc.gpsimd.collective_compute("AllGather", AO.bypass,
                                                 replica_groups=rg,
                                                 ins=[ag_in[:]], outs=[table[:]])
                else:
                    nc.scalar.copy(out=agb3[:, :, 0:D], in_=nh3)
                    nc.sync.dma_start(out=ag_in[:], in_=agb[:])
                    nc.gpsimd.collective_compute("AllGather", AO.bypass,
                                                 replica_groups=rg,
                                                 ins=[ag_in[:]], outs=[table[:]])
                scale = nrm2 if hop < HOPS else nrm
                acc = None
                for (h, tile0, ntl) in pp['calls']:
                    g = gp.tile([P, CHUNK_TILES, ROW], bf16, tag="g")
                    base = table[HALF:, :] if h == HI else (table[:HALF, :] if N > HALF else table[:, :])
                    q = qi % NQ
                    qi += 1
                    nc.gpsimd.dma_gather(
                        out_ap=g[:, :ntl, :], in_ap=base,
                        idxs_ap=idxt[:, tile0 * 8:(tile0 + ntl) * 8],
                        num_idxs=ntl * P, num_idxs_reg=ntl * P, elem_size=ROW,
                        single_packet=False, queue_num=q)
                    ohb = op_.tile([P, CHUNK_TILES * P], bf16, tag="oh")
                    nc.vector.tensor_tensor(
                        out=ohb[:, :ntl * P].rearrange("p (t n) -> p t n", n=P),
                        in0=iot[:].rearrange("p (o n) -> p o n", o=1)
                            .to_broadcast([P, ntl, P]),
                        in1=dstl[:, tile0:tile0 + ntl]
                            .rearrange("p (t o) -> p t o", o=1)
                            .to_broadcast([P, ntl, P]),
                        op=AO.is_equal)
                    for j in range(ntl):
                        tglob = tile0 + j
                        th, b, first, last = pp['tile_meta'][tglob]
                        if first:
                            acc = ps.tile([P, D], f32, tag="acc", space="PSUM")
                        nc.tensor.matmul(out=acc[:], lhsT=ohb[:, j * P:(j + 1) * P],
                                         rhs=g[:, j, 0:D], start=first, stop=last)
                        if last:
                            if th == LO and pp['THI'] > 0:
                                nc.vector.tensor_copy(out=prt[:, bs(b)], in_=acc[:])
                            else:
                                if pp['THI'] > 0:
                                    nc.vector.tensor_tensor(out=nh[:, bs(b)], in0=acc[:],
                                                            in1=prt[:, bs(b)], op=AO.add)
                                else:
                                    nc.vector.tensor_copy(out=nh[:, bs(b)], in_=acc[:])
                                nc.vector.tensor_scalar(out=nh[:, bs(b)], in0=nh[:, bs(b)],
                                                        scalar1=scale[:, b:b + 1],
                                                        scalar2=None, op0=AO.mult)

            # --- nh now holds r = norm * agg. LayerNorm (batched) + FFN.
            r3 = nh[:].rearrange("p (b d) -> p b d", d=D)
            xc = wp.tile([P, NB * D], f32)
            xc3 = xc[:].rearrange("p (b d) -> p b d", d=D)
            sq3 = prt[:].rearrange("p (b d) -> p b d", d=D)  # reuse prt as scratch
            mu = wp.tile([P, NB], f32)
            ssq = wp.tile([P, NB], f32)
            rstd = wp.tile([P, NB], f32)
            X = mybir.AxisListType.X
            nc.vector.tensor_reduce(out=mu[:], in_=r3, axis=X, op=AO.add)
            nc.vector.tensor_scalar(out=mu[:], in0=mu[:], scalar1=1.0 / D, scalar2=None,
                                    op0=AO.mult)
            nc.vector.tensor_tensor(out=xc3, in0=r3,
                                    in1=mu[:].rearrange("p (b o) -> p b o", o=1).to_broadcast([P, NB, D]),
                                    op=AO.subtract)
            nc.vector.tensor_tensor(out=sq3, in0=xc3, in1=xc3, op=AO.mult)
            nc.vector.tensor_reduce(out=ssq[:], in_=sq3, axis=X, op=AO.add)
            nc.vector.tensor_scalar(out=ssq[:], in0=ssq[:], scalar1=1.0 / D, scalar2=None,
                                    op0=AO.mult)
            nc.vector.tensor_scalar(out=ssq[:], in0=ssq[:], scalar1=LN_EPS,
                                    scalar2=None, op0=AO.add)
            nc.scalar.activation(out=ssq[:], in_=ssq[:],
                                 func=mybir.ActivationFunctionType.Sqrt)
            nc.vector.reciprocal(rstd[:], ssq[:])
            # xln = xc * rstd * gamma + beta   (reuse xc buffer)
            nc.vector.tensor_tensor(out=xc3, in0=xc3,
                                    in1=rstd[:].rearrange("p (b o) -> p b o", o=1).to_broadcast([P, NB, D]),
                                    op=AO.mult)
            nc.vector.tensor_tensor(out=xc3, in0=xc3,
                                    in1=gmb[:].rearrange("p (o d) -> p o d", o=1).to_broadcast([P, NB, D]),
                                    op=AO.mult)
            nc.vector.tensor_tensor(out=xc3, in0=xc3,
                                    in1=btb[:].rearrange("p (o d) -> p o d", o=1).to_broadcast([P, NB, D]),
                                    op=AO.add)
            # bf16 copy of the LN output for the PE matmuls
            xcb = wp.tile([P, NB * D], bf16)
            nc.scalar.copy(out=xcb[:], in_=xc[:])

            out_own = wp.tile([P, NB * D], f32)
            for b in range(NB):
                xT_ps = psf.tile([D, P], bf16, tag="tr", space="PSUM")
                nc.tensor.transpose(out=xT_ps[:], in_=xcb[:, bs(b)], identity=idn[:])
                xT = op_.tile([D, P], bf16, tag="xT")
                nc.scalar.copy(xT[:], xT_ps[:])
                h1_ps = psf.tile([D, P], f32, tag="h1", space="PSUM")
                nc.tensor.matmul(out=h1_ps[:], lhsT=w1T[:], rhs=xT[:], start=True, stop=True)
                h1 = op_.tile([D, P], bf16, tag="h1s")
                nc.scalar.activation(out=h1[:], in_=h1_ps[:],
                                     func=mybir.ActivationFunctionType.Relu,
                                     bias=b1[:, 0:1])
                ff_ps = psf.tile([P, D], f32, tag="ff", space="PSUM")
                nc.tensor.matmul(out=ff_ps[:], lhsT=h1[:], rhs=w2T[:], start=True, stop=True)
                nc.vector.tensor_tensor(out=out_own[:, bs(b)], in0=ff_ps[:],
                                        in1=nh[:, bs(b)], op=AO.add)
            o3 = out_own[:].rearrange("p (b d) -> p b d", d=D)
            nc.vector.tensor_tensor(out=o3, in0=o3,
                                    in1=feat[:].rearrange("p (b d) -> p b d", d=D), op=AO.add)
            nc.vector.tensor_tensor(out=o3, in0=o3,
                                    in1=b2b[:].rearrange("p (o d) -> p o d", o=1).to_broadcast([P, NB, D]),
                                    op=AO.add)
            nc.sync.dma_start(out=t_out[:], in_=out_own[:])
            nc.sync.dma_start(out=t_r[:], in_=nh[:])
    nc.compile()
    return nc


def kernel(features, edge_src, edge_dst, w1, b1, w2, b2, gamma, beta):
    from concourse import bass_utils
    import ml_dtypes
    bf = ml_dtypes.bfloat16
    features = np.asarray(features, np.float32)
    edge_src = np.asarray(edge_src, np.int32)
    edge_dst = np.asarray(edge_dst, np.int32)
    N = features.shape[0]
    NPC = N // NC
    NB = NPC // P

    deg = np.bincount(edge_dst, minlength=N).astype(np.float32)
    norm = 1.0 / np.sqrt(np.maximum(deg, 1.0))

    import hashlib
    h = hashlib.sha1()
    h.update(edge_src.tobytes())
    h.update(edge_dst.tobytes())
    h.update(str(N).encode())
    key = h.hexdigest()
    if key not in _CACHE:
        pp = _preprocess(N, edge_src, edge_dst, norm)
        ncb = _build(N, pp)
        _CACHE[key] = (pp, ncb)
    pp, ncb = _CACHE[key]

    # per-core host arrays
    iota_np = np.tile(np.arange(P, dtype=np.float32), (P, 1)).astype(bf)
    ident_np = np.eye(P, dtype=np.float32).astype(bf)
    w1T_np = np.ascontiguousarray(np.asarray(w1, np.float32).T).astype(bf)
    w2T_np = np.ascontiguousarray(np.asarray(w2, np.float32).T).astype(bf)
    b1_np = np.asarray(b1, np.float32).reshape(D, 1)
    b2b_np = np.tile(np.asarray(b2, np.float32)[None, :], (P, 1))
    gam_np = np.tile(np.asarray(gamma, np.float32)[None, :], (P, 1))
    bet_np = np.tile(np.asarray(beta, np.float32)[None, :], (P, 1))

    in_maps = []
    for k in range(NC):
        nm = pp['nodemap'][k]
        fo = features[nm].reshape(NB, P, D).transpose(1, 0, 2) \
            .reshape(P, NB * D).copy()
        no = norm[nm].reshape(NB, P).T.copy()
        t0 = np.zeros((P, NB, ROW), np.float32)
        t0[:, :, :D] = (fo * np.repeat(no, D, 1)).reshape(P, NB, D)
        in_maps.append({
            "feat": fo, "tab0": t0.reshape(P, NB * ROW).astype(bf),
            "idx16": pp['idx16'][k], "dstl": pp['dstl'][k].astype(bf),
            "normv": no, "norm2v": (no * no), "iotar": iota_np, "ident": ident_np,
            "w1T": w1T_np, "w2T": w2T_np, "b1c": b1_np, "b2b": b2b_np,
            "gamb": gam_np, "betb": bet_np,
        })

    trace = os.environ.get("GCN_TRACE", "0") == "1"
    res = bass_utils.run_bass_kernel_spmd(ncb, in_maps, core_ids=list(range(NC)),
                                          trace=trace)
    if trace and res.exec_time_ns is not None:
        print(f"HW exec time: {res.exec_time_ns} ns")

    out = np.empty((N, D), np.float32)
    r = np.empty((N, D), np.float32)
    for k in range(NC):
        nm = pp['nodemap'][k]
        o = res.results[k]["outp"].reshape(P, NB, D).transpose(1, 0, 2).reshape(NPC, D)
        rr = res.results[k]["routp"].reshape(P, NB, D).transpose(1, 0, 2).reshape(NPC, D)
        out[nm] = o
        r[nm] = rr
    return (out, r)


# revision 23
# speedup vs baseline: 3.7986x; 1.3963x over previous
"""GCN block (3-hop symmetric-normalized propagation + LN/FFN/residual) on 8 trn2 cores.

Strategy:
  - Nodes sharded 8 ways (8192/core), edges partitioned by destination core.
  - Per core, destinations grouped into 64 blocks of 128 nodes; each block's
    edge list split by gather-table half (row < 32768 vs >= 32768) so that
    dma_gather's int16 indices always fit; both halves accumulate into the
    same PSUM result via a two-pass partial-sum scheme.
  - Gather table is bf16 with 256B rows (64 feats + 64 pad) in shared DRAM.
    Messages gathered with nc.gpsimd.dma_gather in prepare_only mode,
    round-robined over all 4 SWDGE queues (4 independent DMA rings), each
    triggered immediately; scatter-add done with DVE-built bf16 one-hot
    matrices and bf16 PE matmuls accumulating in fp32 PSUM.
  - Between hops the updated owned block is cast to bf16 and AllGathered into
    every core's table (collective overlaps little; it is cheap).
  - LN in fp32 on DVE; FFN matmuls in bf16 (x @ w1.T relu @ w2.T), residuals
    fp32, outputs (out, r).
"""
import sys
sys.path.insert(0, '/opt/trn_rl_repo')
import os
import numpy as np

NC = 8          # cores
P = 128         # partitions
D = 64          # feature dim
ROW = 128       # padded bf16 row elems (256B)
HOPS = 3
LN_EPS = 1e-5
HALF = 32768    # int16-safe table split
CHUNK_TILES = 16  # 2048 edges per dma_gather call
NQ = 4          # SWDGE queues
LO, HI = 0, 1

_CACHE = {}


def _dma_gather_narrow(eng, out_ap, in_ap, idxs_ap, num_idxs, num_idxs_reg,
                       elem_size, elem_step, queue_num):
    """nc.gpsimd.dma_gather with the 256B-payload restriction relaxed:
    payload = elem_size elements per row, rows at elem_step stride (the
    stride must stay 256B-aligned; the SDMA descriptor just carries a
    byte length, and the ucode handles arbitrary packet_bytes)."""
    from concourse import mybir, ap_utils
    eng._assert_queue_num(queue_num)
    assert idxs_ap.dtype == mybir.dt.int16
    assert in_ap.dtype == out_ap.dtype
    assert ap_utils.ap_is_contiguous(out_ap.ap[1:])
    assert ap_utils.ap_is_contiguous(idxs_ap.ap[1:])
    assert in_ap.ap[-1][1] == out_ap.ap[-1][1] == elem_size
    assert out_ap.ap[0][1] * out_ap.ap[1][1] == num_idxs
    assert in_ap.ap[0][0] == elem_step
    stride_bytes = elem_step * mybir.dt.size(in_ap.dtype)
    assert stride_bytes % 256 == 0
    stride_bytes_256 = stride_bytes // 256
    assert 0 < stride_bytes_256 < 256
    _in_ap = eng.lower_ap_dma(in_ap, for_custom_bir_dma=True)
    _idxs_ap = eng.lower_ap(idxs_ap)
    _out_ap = eng.lower_ap(out_ap)
    inst = eng.add_instruction(
        mybir.InstDMAGatherAnt(
            name=eng.bass.get_next_instruction_name(),
            ins=[*_in_ap, _idxs_ap,
                 eng.lower_val_access(eng.to_reg(num_idxs_reg))],
            outs=[_out_ap],
            transpose=False,
            num_idxs=num_idxs,
            elem_size=elem_size,
            stride_bytes_256=stride_bytes_256,
            gen_mode=0,
            single_packet=False,
            queue_num=queue_num,
            sbuf_tokens_per_rank=0,
            sbuf_free_dim_per_rank=0,
            sbuf_free_dim_pad_per_rank=0,
            sbuf_byte_offset=0,
        )
    )
    return inst.annotate(f"swdge_q={queue_num}")


def _rebalance(N, edge_src, edge_dst):
    """Assign nodes to (core, block) bins balancing per-bin (lo, hi) edge
    counts, so per-block tile counts hit their floor. A node keeps its table
    half (orig id < HALF <=> cores 0..NC/2), so every edge's half bit is
    invariant under the reassignment. Returns (k_of, b_of, p_of, nodemap)."""
    NPC = N // NC
    NB = NPC // P
    if N <= HALF:
        groups = [(0, N, 0, NC)]
    else:
        groups = [(0, HALF, 0, NC // 2), (HALF, N, NC // 2, NC)]
    lo = np.bincount(edge_dst[edge_src < HALF], minlength=N).astype(np.int64)
    hi = np.bincount(edge_dst[edge_src >= HALF], minlength=N).astype(np.int64)
    k_of = np.empty(N, np.int64)
    b_of = np.empty(N, np.int64)
    p_of = np.empty(N, np.int64)
    nodemap = np.empty((NC, NPC), np.int64)
    for (n0, n1, c0, c1) in groups:
        ncores = c1 - c0
        nbins = ncores * NB
        nodes = np.arange(n0, n1)
        order = nodes[np.argsort(-(lo[nodes] + hi[nodes]), kind='stable')]
        binlo = np.zeros(nbins, np.int64)
        binhi = np.zeros(nbins, np.int64)
        slots = np.zeros(nbins, np.int64)

        def place(n, g):
            k = c0 + g // NB
            b = g % NB
            p = slots[g]
            k_of[n] = k
            b_of[n] = b
            p_of[n] = p
            nodemap[k, b * P + p] = n
            binlo[g] += lo[n]
            binhi[g] += hi[n]
            slots[g] += 1

        # The heaviest nodes are concentrated into the last NDB blocks ("dump"
        # bins): slot packing is exact (128 nodes per bin), so spare capacity
        # for the other blocks only exists if dump blocks absorb extra degree.
        NDB = 1
        ndump = ncores * P * NDB
        for i, n in enumerate(order[:ndump]):
            place(n, (i % (ncores * NDB)) // NDB * NB + (NB - 1 - i % NDB))
        # Remaining nodes: dealt one-per-bin per round (slot counts stay
        # exactly equal); within a round the bin furthest below the running
        # lo (or hi, alternating) average receives the node richest in it.
        live = np.nonzero(~((np.arange(nbins) % NB) >= (NB - NDB)))[0]
        rest = order[ndump:]
        nlive = len(live)
        for r in range(P):
            chunk = rest[r * nlive:(r + 1) * nlive]
            key = lo[chunk] if r % 2 == 0 else hi[chunk]
            nsort = chunk[np.argsort(-(2 * key + lo[chunk] + hi[chunk]),
                                     kind='stable')]
            bload = binlo[live] if r % 2 == 0 else binhi[live]
            bsort = live[np.argsort(bload + 0.25 * (binlo + binhi)[live],
                                    kind='stable')]
            for n, g in zip(nsort, bsort):
                place(n, int(g))
    return k_of, b_of, p_of, nodemap


def _preprocess(N, edge_src, edge_dst, norm):
    """Partition/pad edges; returns per-core arrays + shared tile schedule."""
    NPC = N // NC          # nodes per core
    NB = NPC // P          # dst blocks per core
    E = edge_src.shape[0]

    k_of, b_of, p_of, nodemap = _rebalance(N, edge_src, edge_dst)
    s = edge_src.astype(np.int64)
    d = edge_dst.astype(np.int64)
    # table row of node n (p-major per core): k*NPC + p*NB + b
    rho = k_of[s] * NPC + p_of[s] * NB + b_of[s]
    k_d = k_of[d]
    b_d = b_of[d]
    dl_d = p_of[d]
    half = (rho >= HALF).astype(np.int64)

    # group edges by (core, block, half)
    key = ((k_d * NB + b_d) * 2 + half).astype(np.int64)
    order = np.argsort(key, kind='stable')
    cnt = np.bincount(key, minlength=NC * NB * 2).reshape(NC, NB, 2)
    # equalized tile counts across cores (SPMD: one program)
    T = np.maximum(1, (cnt.max(axis=0) + P - 1) // P)  # [NB, 2] tiles
    if N <= HALF:
        T[:, HI] = 0
    TLO, THI = int(T[:, LO].sum()), int(T[:, HI].sum())
    TT = TLO + THI
    EP = TT * P  # padded edges per core

    rho_s = rho[order]
    dl_s = dl_d[order]
    starts = np.zeros(NC * NB * 2 + 1, np.int64)
    np.cumsum(np.bincount(key, minlength=NC * NB * 2), out=starts[1:])

    idx_all = np.zeros((NC, EP), np.int64)
    dstl_all = np.full((NC, EP), 200.0, np.float32)  # sentinel -> one-hot row of zeros
    # per-core streams: [all lo tiles b0..b63][all hi tiles b0..b63]
    tile_meta = []  # shared schedule: (half, block, first, last)
    for h in (LO, HI):
        for b in range(NB):
            for t in range(T[b, h]):
                tile_meta.append((h, b, t == 0, t == T[b, h] - 1))
    off_h = [0, TLO * P]
    for k in range(NC):
        for h in (LO, HI):
            pos = off_h[h]
            for b in range(NB):
                g = (k * NB + b) * 2 + h
                c = int(starts[g + 1] - starts[g])
                sl = slice(starts[g], starts[g + 1])
                idx_all[k, pos:pos + c] = rho_s[sl] - (HALF if h else 0)
                dstl_all[k, pos:pos + c] = dl_s[sl]
                # pad rows gather row 0 of the half; dstl stays sentinel
                pos += T[b, h] * P
    # wrapped int16 index layout: edge i -> [i%16, i//16], replicated to 128 partitions
    idx16 = idx_all.reshape(NC, EP // 16, 16).transpose(0, 2, 1).astype(np.int16)
    idx16 = np.tile(idx16, (1, 8, 1))  # [NC, 128, EP//16]
    dstl = dstl_all.reshape(NC, TT, P).transpose(0, 2, 1).copy()  # [NC, 128, TT]

    # gather-call schedule (shared): per half, runs of <=CHUNK_TILES tiles
    calls = []  # (half, tile0, ntiles)
    for h, t0, tn in ((LO, 0, TLO), (HI, TLO, THI)):
        t = 0
        while t < tn:
            n = min(CHUNK_TILES, tn - t)
            calls.append((h, t0 + t, n))
            t += n
    return dict(NPC=NPC, NB=NB, TT=TT, TLO=TLO, THI=THI, tile_meta=tile_meta,
                calls=calls, idx16=idx16, dstl=dstl, nodemap=nodemap)


def _build(N, pp):
    from concourse import bass, bacc, tile, mybir
    NPC, NB, TT = pp['NPC'], pp['NB'], pp['TT']
    EP = TT * P
    f32, bf16, i16 = mybir.dt.float32, mybir.dt.bfloat16, mybir.dt.int16
    AO = mybir.AluOpType

    nc = bacc.Bacc("TRN2", target_bir_lowering=False, debug=False, num_devices=NC,
                   num_swdge_queues=NQ)
    # inputs (per-core)
    t_feat = nc.dram_tensor("feat", [P, NB * D], f32, kind="ExternalInput")
    t_tab0 = nc.dram_tensor("tab0", [P, NB * ROW], bf16, kind="ExternalInput")
    t_idx = nc.dram_tensor("idx16", [P, EP // 16], i16, kind="ExternalInput")
    t_dstl = nc.dram_tensor("dstl", [P, TT], bf16, kind="ExternalInput")
    t_norm = nc.dram_tensor("normv", [P, NB], f32, kind="ExternalInput")
    t_norm2 = nc.dram_tensor("norm2v", [P, NB], f32, kind="ExternalInput")
    t_iota = nc.dram_tensor("iotar", [P, CHUNK_TILES * P], bf16, kind="ExternalInput")
    t_ident = nc.dram_tensor("ident", [P, P], bf16, kind="ExternalInput")
    t_w1T = nc.dram_tensor("w1T", [D, D], bf16, kind="ExternalInput")
    t_w2T = nc.dram_tensor("w2T", [D, D], bf16, kind="ExternalInput")
    t_b1 = nc.dram_tensor("b1c", [D, 1], f32, kind="ExternalInput")
    t_b2 = nc.dram_tensor("b2b", [P, D], f32, kind="ExternalInput")
    t_gam = nc.dram_tensor("gamb", [P, D], f32, kind="ExternalInput")
    t_bet = nc.dram_tensor("betb", [P, D], f32, kind="ExternalInput")
    # outputs
    t_out = nc.dram_tensor("outp", [P, NB * D], f32, kind="ExternalOutput")
    t_r = nc.dram_tensor("routp", [P, NB * D], f32, kind="ExternalOutput")

    with tile.TileContext(nc) as tc:
        with tc.tile_pool(name="const", bufs=1) as cp, \
             tc.tile_pool(name="work", bufs=1) as wp, \
             tc.tile_pool(name="g", bufs=12) as gp, \
             tc.tile_pool(name="oh", bufs=5) as op_, \
             tc.tile_pool(name="ps", bufs=3, space="PSUM") as ps, \
             tc.tile_pool(name="psf", bufs=1, space="PSUM") as psf, \
             tc.tile_pool(name="dram", bufs=1, space="DRAM") as dr:

            # --- load constants / inputs into SBUF
            feat = cp.tile([P, NB * D], f32)
            idxt = cp.tile([P, EP // 16], i16)
            dstl = cp.tile([P, TT], bf16)
            nrm = cp.tile([P, NB], f32)
            nrm2 = cp.tile([P, NB], f32)
            iot = cp.tile([P, CHUNK_TILES * P], bf16)
            idn = cp.tile([P, P], bf16)
            w1T = cp.tile([D, D], bf16)
            w2T = cp.tile([D, D], bf16)
            b1 = cp.tile([D, 1], f32)
            b2b = cp.tile([P, D], f32)
            gmb = cp.tile([P, D], f32)
            btb = cp.tile([P, D], f32)
            for tl, th in ((feat, t_feat), (idxt, t_idx), (dstl, t_dstl),
                           (nrm, t_norm), (nrm2, t_norm2), (iot, t_iota),
                           (idn, t_ident), (w1T, t_w1T), (w2T, t_w2T),
                           (b1, t_b1), (b2b, t_b2), (gmb, t_gam), (btb, t_bet)):
                nc.sync.dma_start(out=tl[:], in_=th[:])

            nh = wp.tile([P, NB * D], f32)     # owned block, premultiplied
            prt = wp.tile([P, NB * D], f32)    # partial agg between lo/hi passes
            agb = wp.tile([P, NB * ROW], bf16)  # bf16 padded staging for table
            nc.vector.memset(agb[:], 0.0)
            ag_in = dr.tile([P, NB * ROW], bf16)
            tables = []
            for hh in range(HOPS):
                tbl = dr.tile([N, ROW], bf16, addr_space="Shared", tag=f"table{hh}")
                tables.append(tbl)

            def bs(b):
                return slice(b * D, (b + 1) * D)

            agb3 = agb[:].rearrange("p (b c) -> p b c", c=ROW)
            nh3 = nh[:].rearrange("p (b d) -> p b d", d=D)

            rg = [list(range(NC))]
            qi = 0
            for hop in range(1, HOPS + 1):
                # publish owned block -> table (all cores), bf16 padded rows.
                # hop 1's slice (norm*features) is precomputed on the host.
                table = tables[hop - 1]
                if hop == 1:
                    nc.sync.dma_start(out=ag_in[:], in_=t_tab0[:])
                    nc.gpsimd.collective_compute("AllGather", AO.bypass,
                                                 replica_groups=rg,
                                                 ins=[ag_in[:]], outs=[table[:]])
                else:
                    for b in range(NB):
                        nc.scalar.activation(
                            out=agb3[:, b, 0:D], in_=nh3[:, b],
                            func=mybir.ActivationFunctionType.Copy,
                            scale=nrm2[:, b:b + 1])
                    nc.sync.dma_start(out=ag_in[:], in_=agb[:])
                    nc.gpsimd.collective_compute("AllGather", AO.bypass,
                                                 replica_groups=rg,
                                                 ins=[ag_in[:]], outs=[table[:]])
                scale = nrm2 if hop < HOPS else nrm
                acc = None
                for (h, tile0, ntl) in pp['calls']:
                    g = gp.tile([P, CHUNK_TILES, D], bf16, tag="g")
                    base = table[HALF:, :] if h == HI else (table[:HALF, :] if N > HALF else table[:, :])
                    q = qi % NQ
                    qi += 1
                    _dma_gather_narrow(
                        nc.gpsimd, out_ap=g[:, :ntl, :], in_ap=base[:, 0:D],
                        idxs_ap=idxt[:, tile0 * 8:(tile0 + ntl) * 8],
                        num_idxs=ntl * P, num_idxs_reg=ntl * P, elem_size=D,
                        elem_step=ROW, queue_num=q)
                    ohb = op_.tile([P, CHUNK_TILES * P], bf16, tag="oh")
                    nc.vector.tensor_tensor(
                        out=ohb[:, :ntl * P].rearrange("p (t n) -> p t n", n=P),
                        in0=iot[:, :ntl * P].rearrange("p (t n) -> p t n", n=P),
                        in1=dstl[:, tile0:tile0 + ntl]
                            .rearrange("p (t o) -> p t o", o=1)
                            .to_broadcast([P, ntl, P]),
                        op=AO.is_equal)
                    for j in range(ntl):
                        tglob = tile0 + j
                        th, b, first, last = pp['tile_meta'][tglob]
                        if first:
                            acc = ps.tile([P, D], f32, tag="acc", space="PSUM")
                        nc.tensor.matmul(out=acc[:], lhsT=ohb[:, j * P:(j + 1) * P],
                                         rhs=g[:, j, :], start=first, stop=last)
                        if last:
                            if th == LO and pp['THI'] > 0:
                                nc.scalar.copy(out=prt[:, bs(b)], in_=acc[:])
                            elif pp['THI'] > 0:
                                nc.vector.tensor_tensor(out=nh[:, bs(b)], in0=acc[:],
                                                        in1=prt[:, bs(b)], op=AO.add)
                            else:
                                nc.scalar.copy(out=nh[:, bs(b)], in_=acc[:])
                            if hop == HOPS and (th == HI or pp['THI'] == 0):
                                nc.scalar.activation(
                                    out=nh[:, bs(b)], in_=nh[:, bs(b)],
                                    func=mybir.ActivationFunctionType.Copy,
                                    scale=scale[:, b:b + 1])

            # --- nh now holds r = norm * agg. LayerNorm (batched) + FFN.
            r3 = nh[:].rearrange("p (b d) -> p b d", d=D)
            xc = wp.tile([P, NB * D], f32)
            xc3 = xc[:].rearrange("p (b d) -> p b d", d=D)
            sq3 = prt[:].rearrange("p (b d) -> p b d", d=D)  # reuse prt as scratch
            mu = wp.tile([P, NB], f32)
            ssq = wp.tile([P, NB], f32)
            rstd = wp.tile([P, NB], f32)
            X = mybir.AxisListType.X
            nc.vector.tensor_reduce(out=mu[:], in_=r3, axis=X, op=AO.add)
            nc.vector.tensor_scalar(out=mu[:], in0=mu[:], scalar1=1.0 / D, scalar2=None,
                                    op0=AO.mult)
            nc.vector.tensor_tensor(out=xc3, in0=r3,
                                    in1=mu[:].rearrange("p (b o) -> p b o", o=1).to_broadcast([P, NB, D]),
                                    op=AO.subtract)
            nc.vector.tensor_tensor(out=sq3, in0=xc3, in1=xc3, op=AO.mult)
            nc.vector.tensor_reduce(out=ssq[:], in_=sq3, axis=X, op=AO.add)
            nc.vector.tensor_scalar(out=ssq[:], in0=ssq[:], scalar1=1.0 / D, scalar2=None,
                                    op0=AO.mult)
            nc.vector.tensor_scalar(out=ssq[:], in0=ssq[:], scalar1=LN_EPS,
                                    scalar2=None, op0=AO.add)
            nc.scalar.activation(out=ssq[:], in_=ssq[:],
                                 func=mybir.ActivationFunctionType.Sqrt)
            nc.vector.reciprocal(rstd[:], ssq[:])
            # xln = xc * rstd * gamma + beta   (reuse xc buffer)
            nc.vector.tensor_tensor(out=xc3, in0=xc3,
                                    in1=rstd[:].rearrange("p (b o) -> p b o", o=1).to_broadcast([P, NB, D]),
                                    op=AO.mult)
            nc.vector.tensor_tensor(out=xc3, in0=xc3,
                                    in1=gmb[:].rearrange("p (o d) -> p o d", o=1).to_broadcast([P, NB, D]),
                                    op=AO.mult)
            nc.vector.tensor_tensor(out=xc3, in0=xc3,
                                    in1=btb[:].rearrange("p (o d) -> p o d", o=1).to_broadcast([P, NB, D]),
                                    op=AO.add)
            # bf16 copy of the LN output for the PE matmuls
            xcb = wp.tile([P, NB * D], bf16)
            nc.scalar.copy(out=xcb[:], in_=xc[:])

            out_own = wp.tile([P, NB * D], f32)
            for b in range(NB):
                xT_ps = psf.tile([D, P], bf16, tag="tr", space="PSUM")
                nc.tensor.transpose(out=xT_ps[:], in_=xcb[:, bs(b)], identity=idn[:])
                xT = op_.tile([D, P], bf16, tag="xT")
                nc.scalar.copy(xT[:], xT_ps[:])
                h1_ps = psf.tile([D, P], f32, tag="h1", space="PSUM")
                nc.tensor.matmul(out=h1_ps[:], lhsT=w1T[:], rhs=xT[:], start=True, stop=True)
                h1 = op_.tile([D, P], bf16, tag="h1s")
                nc.scalar.activation(out=h1[:], in_=h1_ps[:],
                                     func=mybir.ActivationFunctionType.Relu,
                                     bias=b1[:, 0:1])
                ff_ps = psf.tile([P, D], f32, tag="ff", space="PSUM")
                nc.tensor.matmul(out=ff_ps[:], lhsT=h1[:], rhs=w2T[:], start=True, stop=True)
                nc.vector.tensor_tensor(out=out_own[:, bs(b)], in0=ff_ps[:],
                                        in1=nh[:, bs(b)], op=AO.add)
            o3 = out_own[:].rearrange("p (b d) -> p b d", d=D)
            nc.vector.tensor_tensor(out=o3, in0=o3,
                                    in1=feat[:].rearrange("p (b d) -> p b d", d=D), op=AO.add)
            nc.vector.tensor_tensor(out=o3, in0=o3,
                                    in1=b2b[:].rearrange("p (o d) -> p o d", o=1).to_broadcast([P, NB, D]),
                                    op=AO.add)
            nc.sync.dma_start(out=t_out[:], in_=out_own[:])
            nc.sync.dma_start(out=t_r[:], in_=nh[:])
    nc.compile()
    return nc


def kernel(features, edge_src, edge_dst, w1, b1, w2, b2, gamma, beta):
    from concourse import bass_utils
    import ml_dtypes
    bf = ml_dtypes.bfloat16
    features = np.asarray(features, np.float32)
    edge_src = np.asarray(edge_src, np.int32)
    edge_dst = np.asarray(edge_dst, np.int32)
    N = features.shape[0]
    NPC = N // NC
    NB = NPC // P

    deg = np.bincount(edge_dst, minlength=N).astype(np.float32)
    norm = 1.0 / np.sqrt(np.maximum(deg, 1.0))

    import hashlib
    h = hashlib.sha1()
    h.update(edge_src.tobytes())
    h.update(edge_dst.tobytes())
    h.update(str(N).encode())
    key = h.hexdigest()
    if key not in _CACHE:
        pp = _preprocess(N, edge_src, edge_dst, norm)
        ncb = _build(N, pp)
        _CACHE[key] = (pp, ncb)
    pp, ncb = _CACHE[key]

    # per-core host arrays
    iota_np = np.tile(np.arange(P, dtype=np.float32), (P, CHUNK_TILES)).astype(bf)
    ident_np = np.eye(P, dtype=np.float32).astype(bf)
    w1T_np = np.ascontiguousarray(np.asarray(w1, np.float32).T).astype(bf)
    w2T_np = np.ascontiguousarray(np.asarray(w2, np.float32).T).astype(bf)
    b1_np = np.asarray(b1, np.float32).reshape(D, 1)
    b2b_np = np.tile(np.asarray(b2, np.float32)[None, :], (P, 1))
    gam_np = np.tile(np.asarray(gamma, np.float32)[None, :], (P, 1))
    bet_np = np.tile(np.asarray(beta, np.float32)[None, :], (P, 1))

    in_maps = []
    for k in range(NC):
        nm = pp['nodemap'][k]
        fo = features[nm].reshape(NB, P, D).transpose(1, 0, 2) \
            .reshape(P, NB * D).copy()
        no = norm[nm].reshape(NB, P).T.copy()
        t0 = np.zeros((P, NB, ROW), np.float32)
        t0[:, :, :D] = (fo * np.repeat(no, D, 1)).reshape(P, NB, D)
        in_maps.append({
            "feat": fo, "tab0": t0.reshape(P, NB * ROW).astype(bf),
            "idx16": pp['idx16'][k], "dstl": pp['dstl'][k].astype(bf),
            "normv": no, "norm2v": (no * no), "iotar": iota_np, "ident": ident_np,
            "w1T": w1T_np, "w2T": w2T_np, "b1c": b1_np, "b2b": b2b_np,
            "gamb": gam_np, "betb": bet_np,
        })

    trace = os.environ.get("GCN_TRACE", "0") == "1"
    res = bass_utils.run_bass_kernel_spmd(ncb, in_maps, core_ids=list(range(NC)),
                                          trace=trace)
    if trace and res.exec_time_ns is not None:
        print(f"HW exec time: {res.exec_time_ns} ns")

    out = np.empty((N, D), np.float32)
    r = np.empty((N, D), np.float32)
    for k in range(NC):
        nm = pp['nodemap'][k]
        o = res.results[k]["outp"].reshape(P, NB, D).transpose(1, 0, 2).reshape(NPC, D)
        rr = res.results[k]["routp"].reshape(P, NB, D).transpose(1, 0, 2).reshape(NPC, D)
        out[nm] = o
        r[nm] = rr
    return (out, r)
